# revision 1
# baseline (speedup 1.0000x reference)
"""Trainium2 Bass kernel for nn_CategoryAlign_Module (pooling / cross Pearson).

Math (see reference):
  for each stream s in {1,2}:
    vec_b[k,c]  = sum_p preds[b,k,p] * feats[b,c,p] / sum_p preds[b,k,p]
    ctx_b[k,c]  = vec_b[k,c] / max(||vec_b[:,c]||_2, 1e-12)      (norm over K)
    ctx[k,c]    = mean_b ctx_b[k,c]
  out = pearson(ctx1, ctx2)   (center+normalize rows over C, then ctx1 @ ctx2^T)

Distribution: data-parallel over the batch dim, one batch element per
NeuronCore (B=8, 8 cores).  Each core computes its local normalized
contexts, the tiny [19,512] payload is AllReduce-summed across the 8
cores (Pearson is invariant to the 1/B scale, so the mean's division is
skipped), and every core redundantly computes the replicated [19,19]
correlation.

Per-core pipeline (all big work, bf16 compute / fp32 accumulate):
  - preds arrive host-relayouted as [128, 128*19] so that chunk i's
    columns are the stationary matmul operand P^T[i*128:(i+1)*128, :19]
  - feats stream in as [128, 2048] slabs (1 MB DMAs, fp32->bf16 cast in
    the DMA), are transposed 128x128 at a time on the TensorEngine
    (8 transposes packed per PSUM bank), copied to SBUF, and contracted
    against the preds chunks into a PSUM accumulator [19, 256+1].
"""

import sys

sys.path.insert(0, "/opt/trn_rl_repo")

import numpy as np

import concourse.bass as bass  # noqa: F401  (import order matters)
import concourse.bacc as bacc
import concourse.tile as tile
import concourse.mybir as mybir
from concourse import bass_utils, bass2jax

B, K, C, H, W = 8, 19, 256, 128, 128
P = H * W            # 16384 spatial positions
NCHUNK = P // 128    # 128 contraction chunks
SLAB = 4096          # spatial positions per feats DMA slab (2 MB fp32 reads)
NSLAB = P // SLAB    # 4
QUAD = 4             # p-chunks staged per PSUM bank (4 * 256 bf16 = 1 bank)
N_CORES = 8
EPS = 1e-12

F32 = mybir.dt.float32
BF16 = mybir.dt.bfloat16


def build_body(nc, tc, pret_d, feats_d, ident_d, identf_d, out_d, n_cores,
               nslab=NSLAB, feat=frozenset()):
    """Emit the per-core program.

    pret_d:  2 DRAM APs [128, NCHUNK*K] bf16 (preds, spatial-major relayout)
    feats_d: 2 DRAM APs [C, P] fp32
    ident_d: [128, 128] bf16 identity, identf_d: [K, K] fp32 identity
    out_d:   [K, K] fp32 output
    """
    mult = mybir.AluOpType.mult
    add = mybir.AluOpType.add
    GK = 4          # contraction chunks per mask-sum matmul group
    CCW = C + 1     # per-stream collective payload: [ctx | rowmean]

    with tc.tile_pool(name="persist", bufs=1) as PP, \
         tc.tile_pool(name="acc", bufs=1, space="PSUM") as PA, \
         tc.tile_pool(name="tailp", bufs=1, space="PSUM") as TLP, \
         tc.tile_pool(name="dram", bufs=1, space="DRAM") as DP:

        # --- constants ---
        id_bf = PP.tile([128, 128], BF16, name="id_bf")
        nc.sync.dma_start(id_bf[:], ident_d[:])            # ident arrives bf16
        id_f = PP.tile([K, K], F32, name="id_f")
        nc.sync.dma_start(id_f[:], identf_d[:])
        ones_col = PP.tile([128, 1], BF16, name="ones_col")
        nc.vector.memset(ones_col[:], 1.0)
        ones19 = PP.tile([K, 1], F32, name="ones19")
        nc.vector.memset(ones19[:], 1.0)
        onesrow = PP.tile([1, K], F32, name="onesrow")
        nc.vector.memset(onesrow[:], 1.0)

        # --- preds (spatial-major, pre-cast bf16): HWDGE loads.
        # Stream 1's load is deferred so early HBM bandwidth goes to the
        # first feats slabs.
        PT = []
        for s in (0, 1):
            pt = PP.tile([128, NCHUNK * K], BF16, name=f"PT{s}")
            if s == 0:
                nc.sync.dma_start(pt[:], pret_d[s][:])
            PT.append(pt)

        # --- per-stream accumulators ---
        psum_vec = [PA.tile([K, C], F32, name=f"pvec{s}") for s in (0, 1)]
        psum_srow = [PA.tile([1, GK * K], F32, name=f"psrow{s}")
                     for s in (0, 1)]

        csum = []
        swdge_dmas = []
        bounce = []

        # Slab segmentation: the first 2048 positions go over HWDGE as
        # fp32 (+ DVE cast) to fill the pipeline while the SWDGE Q7 boots;
        # the rest stream as large SWDGE fp32->bf16 cast reads.
        fast_segs = [(o, 512) for o in range(0, 1536, 512)]
        slow_segs0 = [(1536, 512)] + \
            [(o, 2048) for o in range(2048, nslab * SLAB, 2048)]
        segs1 = [(o, 2048) for o in range(0, nslab * SLAB, 2048)]
        if nslab < 2:   # dev bisect shapes
            fast_segs = [(0, 512)]
            slow_segs0 = [(512, nslab * SLAB - 512)]
            segs1 = [(0, nslab * SLAB)]
        last_chunk = nslab * (SLAB // 128) - 1

        with tc.tile_pool(name="fslab", bufs=5) as FP, \
             tc.tile_pool(name="quad", bufs=8) as QP, \
             tc.tile_pool(name="tp", bufs=3, space="PSUM") as TP:
            for s in (0, 1):
                segs = (fast_segs + slow_segs0) if s == 0 else segs1
                # ---- main loop ----
                for si, (base, width) in enumerate(segs):
                    fsl = []
                    for ch in (0, 1):
                        t_ = FP.tile([128, SLAB], BF16, name=f"fsl{ch}")
                        src_ap = feats_d[s][ch * 128:(ch + 1) * 128,
                                            base:base + width]
                        if s == 0 and width == 512:
                            # pipeline-fill fast path: HWDGE fp32 + DVE cast
                            stg = FP.tile([128, 512], F32, name=f"stg{ch}")
                            nc.sync.dma_start(stg[:, 0:width], src_ap)
                            nc.vector.tensor_copy(t_[:, 0:width],
                                                  stg[:, 0:width])
                        else:
                            swdge_dmas.append(
                                nc.gpsimd.dma_start(t_[:, 0:width], src_ap))
                        fsl.append(t_)
                    if s == 0 and si == len(fast_segs) + 1:
                        nc.sync.dma_start(PT[1][:], pret_d[1][:])
                    # mask sums: one matmul per 4 chunks into a [1, 76] row
                    for g in range(width // 512):
                        i0 = base // 128 + g * GK
                        nc.tensor.matmul(
                            psum_srow[s][:],
                            lhsT=ones_col[:],
                            rhs=PT[s][:, i0 * K:(i0 + GK) * K],
                            start=(base == 0 and g == 0),
                            stop=(i0 + GK - 1 == last_chunk))
                    # PE transposes (8 per PSUM bank) + cast-copy + contraction
                    for q in range(width // 512):
                        tp = TP.tile([128, 4 * C], BF16, name="tp")
                        for t in range(4):
                            for ch in (0, 1):
                                idx = t * 2 + ch
                                nc.tensor.matmul(
                                    tp[:, t * C + ch * 128:
                                       t * C + ch * 128 + 128],
                                    lhsT=fsl[ch][:, (q * 4 + t) * 128:
                                                 (q * 4 + t + 1) * 128],
                                    rhs=id_bf[:],
                                    is_transpose=True,
                                    start=(idx == 0), stop=(idx == 7))
                        quad_sb = QP.tile([128, 4 * C], BF16, name="quad_sb")
                        if q % 2 == 0:
                            nc.vector.tensor_copy(quad_sb[:], tp[:])
                        else:
                            nc.scalar.copy(quad_sb[:], tp[:])
                        for t in range(4):
                            i = (base // 128) + q * 4 + t
                            nc.tensor.matmul(
                                psum_vec[s][:],
                                lhsT=PT[s][:, i * K:(i + 1) * K],
                                rhs=quad_sb[:, t * C:(t + 1) * C],
                                start=(i == 0), stop=(i == last_chunk))

                # ---- stream epilogue (stream 0's overlaps stream 1) ----
                srow_sb = PP.tile([1, GK * K], F32, name=f"srow{s}")
                nc.vector.tensor_copy(srow_sb[:], psum_srow[s][:])
                s19 = PP.tile([1, K], F32, name=f"s19_{s}")
                nc.vector.reduce_sum(
                    s19[:], srow_sb[:].rearrange("p (g k) -> p k g", g=GK),
                    axis=mybir.AxisListType.X)
                stmp = TLP.tile([K, 1], F32, name="stmp", tag="tlp")
                nc.tensor.matmul(stmp[:], lhsT=s19[:], rhs=id_f[0:1, 0:1],
                                 is_transpose=True, start=True, stop=True)
                recip = PP.tile([K, 1], F32, name=f"recip{s}")
                nc.vector.reciprocal(recip[:], stmp[:])
                vec_sb = PP.tile([K, C], F32, name=f"vec_sb{s}")
                nc.vector.tensor_scalar_mul(vec_sb[:], psum_vec[s][:],
                                            recip[:])
                sq = PP.tile([K, C], F32, name=f"sq{s}")
                nc.scalar.square(sq[:], vec_sb[:])
                # column sums over K via fp32 matmul with a ones vector
                pn = TLP.tile([1, C], F32, name="pn", tag="tlp")
                nc.tensor.matmul(pn[:], lhsT=ones19[:], rhs=sq[:],
                                 start=True, stop=True)
                # reference clamps the norm at 1e-12; the norm here is
                # O(1e-2) for non-degenerate input, so the clamp is a no-op.
                nsb = PP.tile([1, C], F32, name=f"nsb{s}")
                nc.scalar.sqrt(nsb[:], pn[:])
                rn = PP.tile([1, C], F32, name=f"rn{s}")
                nc.vector.reciprocal(rn[:], nsb[:])
                # broadcast 1/norm to the K partitions (rank-1 matmul)
                bc = TLP.tile([K, C], F32, name="bc", tag="tlp")
                nc.tensor.matmul(bc[:], lhsT=onesrow[:], rhs=rn[:],
                                 start=True, stop=True)
                cc_in = PP.tile([K, CCW], F32, name=f"cc_in{s}")
                nc.vector.tensor_mul(cc_in[:, 0:C], vec_sb[:], bc[:])
                # ship the per-core row-mean in the payload (mean over B and
                # mean over C commute)
                xdum = PP.tile([K, C], F32, name=f"xdum{s}")
                nc.scalar.activation(xdum[:], cc_in[:, 0:C],
                                     mybir.ActivationFunctionType.Copy,
                                     scale=1.0 / C,
                                     accum_out=cc_in[:, C:C + 1])

                # stage the payload for the per-stream AllReduce; the
                # collective instructions are emitted after both streams so
                # they can be ordered AFTER every SWDGE DMA issue (the
                # gpsimd engine blocks on the collective's completion-wait)
                b_in = DP.tile([K, CCW], F32, name=f"b_in{s}")
                b_out = DP.tile([K, CCW], F32, name=f"b_out{s}")
                nc.sync.dma_start(b_in[:], cc_in[:])
                bounce.append((b_in, b_out))

            # ---- the two collectives (stream 0's is hidden under stream
            # 1's compute; both ordered after all SWDGE DMA issues so the
            # completion-wait never stalls the Q7 DMA issuer).  AllGather +
            # local sum beats AllReduce on latency at this payload size. ----
            prev_cc = None
            nT = []
            rinv = []
            for s in (0, 1):
                b_in, b_out = bounce[s]
                cc = nc.gpsimd.collective_compute(
                    "AllReduce", add,
                    replica_groups=[list(range(n_cores))],
                    ins=[b_in.opt()], outs=[b_out.opt()])
                if swdge_dmas:
                    bass._add_dep_helper(
                        cc.ins, swdge_dmas[-1].ins, sync=False,
                        reason="order collective after SWDGE DMA issues")
                if prev_cc is not None:
                    bass._add_dep_helper(
                        cc.ins, prev_cc.ins, sync=False,
                        reason="collectives in stream order")
                prev_cc = cc
                cs = PP.tile([K, CCW], F32, name=f"csum{s}")
                nc.sync.dma_start(cs[:], b_out[:])
                csum.append(cs)

                # ---- side-s Pearson prep (side 0 runs during stream 1 /
                # collective 1; only side 1 trails the last collective) ----
                X = cs[:, 0:C]
                ms = cs[:, C:C + 1]
                xc = PP.tile([K, C], F32, name=f"xc{s}")
                nc.vector.tensor_scalar_sub(xc[:], X, ms)
                xsq = PP.tile([K, C], F32, name=f"xsq{s}")
                ss = PP.tile([K, 1], F32, name=f"ss{s}")
                nc.scalar.activation(xsq[:], xc[:],
                                     mybir.ActivationFunctionType.Square,
                                     accum_out=ss[:])
                sd = PP.tile([K, 1], F32, name=f"sd{s}")
                nc.scalar.sqrt(sd[:], ss[:])
                ri = PP.tile([K, 1], F32, name=f"ri{s}")
                nc.vector.reciprocal(ri[:], sd[:])
                rinv.append(ri)
                xn = PP.tile([K, C], F32, name=f"xn{s}")
                nc.vector.tensor_scalar(xn[:], X, ms, ri[:],
                                        op0=mybir.AluOpType.subtract,
                                        op1=mult)
                # transpose [K, C] -> [C, K] in two 128-wide blocks
                tps = TLP.tile([128, 2 * K], F32, name=f"tps{s}", tag="tlp")
                for h in (0, 1):
                    nc.tensor.matmul(
                        tps[:, h * K:(h + 1) * K],
                        lhsT=xn[:, h * 128:(h + 1) * 128],
                        rhs=id_f[:],
                        is_transpose=True,
                        start=(h == 0), stop=(h == 1))
                nTs = PP.tile([128, 2 * K], F32, name=f"nT{s}")
                nc.vector.tensor_copy(nTs[:], tps[:])
                nT.append(nTs)

            # ---- final correlation ----
            po = TLP.tile([K, K], F32, name="po", tag="tlp")
            for h in (0, 1):
                nc.tensor.matmul(po[:],
                                 lhsT=nT[0][:, h * K:(h + 1) * K],
                                 rhs=nT[1][:, h * K:(h + 1) * K],
                                 start=(h == 0), stop=(h == 1))
            osb = PP.tile([K, K], F32, name="osb")
            nc.vector.tensor_copy(osb[:], po[:])
            nc.sync.dma_start(out_d[:], osb[:])


def build(n_cores=N_CORES, nslab=NSLAB, feat=frozenset()):
    nc = bacc.Bacc("TRN2", target_bir_lowering=False, debug=False,
                   enable_asserts=False, num_devices=n_cores)
    pret_d = [nc.dram_tensor(f"pret{s}", [128, NCHUNK * K], BF16,
                             kind="ExternalInput").ap() for s in (1, 2)]
    feats_d = [nc.dram_tensor(f"feats{s}", [C, P], F32,
                              kind="ExternalInput").ap() for s in (1, 2)]
    ident_d = nc.dram_tensor("ident", [128, 128], BF16, kind="ExternalInput").ap()
    identf_d = nc.dram_tensor("identf", [K, K], F32, kind="ExternalInput").ap()
    out_d = nc.dram_tensor("out", [K, K], F32, kind="ExternalOutput").ap()
    with tile.TileContext(nc) as tc:
        build_body(nc, tc, pret_d, feats_d, ident_d, identf_d, out_d, n_cores,
                   nslab=nslab, feat=feat)
    nc.compile()
    return nc


_NC_CACHE = {}


def _get_nc():
    if "nc" not in _NC_CACHE:
        _NC_CACHE["nc"] = build(N_CORES)
    return _NC_CACHE["nc"]


class Runner:
    """Executes the compiled Bass program on the first `n_cores` jax
    devices via shard_map, with inputs pre-staged on the devices (the
    analog of the native path's input pre-load in run_neff) so all
    cores start the NEFF near-simultaneously."""

    def __init__(self, nc, n_cores):
        import jax
        from jax.experimental.shard_map import shard_map
        from jax.sharding import Mesh, PartitionSpec, NamedSharding

        bass2jax.install_neuronx_cc_hook()
        self.jax = jax
        self.nc = nc
        self.n_cores = n_cores
        assert nc.dbg_addr is None
        partition_name = (nc.partition_id_tensor.name
                          if nc.partition_id_tensor else None)
        in_names, out_names, out_avals = [], [], []
        for alloc in nc.m.functions[0].allocations:
            if not isinstance(alloc, mybir.MemoryLocationSet):
                continue
            name = alloc.memorylocations[0].name
            if alloc.kind == "ExternalInput":
                if name != partition_name:
                    in_names.append(name)
            elif alloc.kind == "ExternalOutput":
                shape = tuple(alloc.tensor_shape)
                dtype = mybir.dt.np(alloc.dtype)
                out_names.append(name)
                out_avals.append(jax.core.ShapedArray(shape, dtype))
        self.param_names = list(in_names)
        n_params = len(in_names)
        full_in_names = list(in_names) + list(out_names)
        if partition_name is not None:
            full_in_names.append(partition_name)
        full_in_names = tuple(full_in_names)
        donate = tuple(range(n_params, n_params + len(out_names)))
        self.out_names = out_names
        self.out_avals = out_avals

        def _body(*args):
            operands = list(args)
            if partition_name is not None:
                operands.append(bass2jax.partition_id_tensor())
            outs = bass2jax._bass_exec_p.bind(
                *operands,
                out_avals=tuple(out_avals),
                in_names=full_in_names,
                out_names=tuple(out_names),
                lowering_input_output_aliases=(),
                sim_require_finite=True,
                sim_require_nnan=True,
                nc=nc,
            )
            return tuple(outs)

        devices = jax.devices()[:n_cores]
        assert len(devices) == n_cores
        self.mesh = Mesh(np.asarray(devices), ("core",))
        in_specs = (PartitionSpec("core"),) * (n_params + len(out_names))
        out_specs = (PartitionSpec("core"),) * len(out_names)
        self.fn = jax.jit(
            shard_map(_body, mesh=self.mesh, in_specs=in_specs,
                      out_specs=out_specs, check_rep=False),
            donate_argnums=donate, keep_unused=True)
        self.sharding = NamedSharding(self.mesh, PartitionSpec("core"))

    def put(self, in_maps):
        concat = [
            np.concatenate([np.asarray(in_maps[c][n])
                            for c in range(self.n_cores)], axis=0)
            for n in self.param_names
        ]
        arrs = [self.jax.device_put(a, self.sharding) for a in concat]
        self.jax.block_until_ready(arrs)
        return arrs

    def zeros(self):
        zs = [self.jax.device_put(
            np.zeros((self.n_cores * a.shape[0], *a.shape[1:]), a.dtype),
            self.sharding) for a in self.out_avals]
        self.jax.block_until_ready(zs)
        return zs

    def exec(self, dev_in):
        outs = self.fn(*dev_in, *self.zeros())
        self.jax.block_until_ready(outs)
        return {
            name: np.asarray(outs[i]).reshape(
                self.n_cores, *self.out_avals[i].shape)
            for i, name in enumerate(self.out_names)
        }


def _get_runner():
    if "runner" not in _NC_CACHE:
        _NC_CACHE["runner"] = Runner(_get_nc(), N_CORES)
    return _NC_CACHE["runner"]


def make_in_maps(preds1, feats1, preds2, feats2):
    import ml_dtypes
    ident = np.eye(128, dtype=ml_dtypes.bfloat16)
    identf = np.eye(K, dtype=np.float32)
    in_maps = []
    for b in range(preds1.shape[0]):
        m = {
            # [K,H,W] -> [W(v), H(u), K] -> [128, 128*19]: chunk u's columns
            # are P^T[u*128:(u+1)*128, :] with the spatial index on partitions
            "pret1": preds1[b].transpose(2, 1, 0).astype(
                ml_dtypes.bfloat16).reshape(128, NCHUNK * K),
            "pret2": preds2[b].transpose(2, 1, 0).astype(
                ml_dtypes.bfloat16).reshape(128, NCHUNK * K),
            "feats1": np.ascontiguousarray(feats1[b]).reshape(C, P),
            "feats2": np.ascontiguousarray(feats2[b]).reshape(C, P),
            "ident": ident,
            "identf": identf,
        }
        in_maps.append(m)
    return in_maps


def kernel(preds1, feats1, preds2, feats2):
    runner = _get_runner()
    in_maps = make_in_maps(preds1, feats1, preds2, feats2)
    dev_in = runner.put(in_maps)
    outs = runner.exec(dev_in)
    return np.asarray(outs["out"][0], dtype=np.float32)



# revision 3
# speedup vs baseline: 1.2594x; 1.2594x over previous
"""Trainium2 Bass kernel for nn_CategoryAlign_Module (pooling / cross Pearson).

Math (see reference):
  for each stream s in {1,2}:
    vec_b[k,c]  = sum_p preds[b,k,p] * feats[b,c,p] / sum_p preds[b,k,p]
    ctx_b[k,c]  = vec_b[k,c] / max(||vec_b[:,c]||_2, 1e-12)      (norm over K)
    ctx[k,c]    = mean_b ctx_b[k,c]
  out = pearson(ctx1, ctx2)   (center+normalize rows over C, then ctx1 @ ctx2^T)

Distribution: data-parallel over the batch dim, one batch element per
NeuronCore (B=8, 8 cores).  Each core computes its local normalized
contexts, the tiny [19,257] payload is AllReduce-summed across the 8
cores (Pearson is invariant to the 1/B scale, so the mean's division is
skipped), and every core redundantly computes the replicated [19,19]
correlation.

Per-core pipeline (bf16 compute / fp32 accumulate):
  - both preds and feats are host-relayouted and host-cast to bf16, so
    the device only moves half the bytes and never transposes:
      preds -> [128, 128*19]  (chunk h: P^T[h*128:(h+1)*128, :19])
      feats -> [128, 128*257] (chunk h: [w, c] block with a fused ones
               column, so psum[:, 256] accumulates the mask sums)
  - bulk DMA is split across the two HWDGE queues (sync + scalar
    engines), 16-chunk segments, so both queues stream concurrently
  - 128 accumulating matmuls per stream produce [19, 257] in PSUM
  - stream 0's AllReduce launches at the halfway point and hides under
    stream 1's DMA; only stream 1's AllReduce + the tiny Pearson
    epilogue are exposed at the tail
"""

import sys

sys.path.insert(0, "/opt/trn_rl_repo")

import numpy as np

import concourse.bass as bass  # noqa: F401  (import order matters)
import concourse.bacc as bacc
import concourse.tile as tile
import concourse.mybir as mybir
from concourse import bass_utils, bass2jax  # noqa: F401

B, K, C, H, W = 8, 19, 256, 128, 128
P = H * W            # 16384 spatial positions
NCHUNK = P // 128    # 128 contraction chunks
CCW = C + 1          # channels + fused ones column (mask sums)
SEGC = 16            # chunks per DMA segment
NSEG = NCHUNK // SEGC
N_CORES = 8

F32 = mybir.dt.float32
BF16 = mybir.dt.bfloat16


def build_body(nc, tc, pret_d, ftr_d, identf_d, out_d, n_cores):
    """Emit the per-core program.

    pret_d: 2 DRAM APs [128, NCHUNK*K] bf16 (preds, spatial-major relayout)
    ftr_d:  2 DRAM APs [128, NCHUNK*CCW] bf16 (feats, spatial-major
            relayout + ones column)
    identf_d: [K, K] fp32 identity (for the tiny Pearson transposes)
    out_d:  [K, K] fp32 output
    """
    add = mybir.AluOpType.add
    mult = mybir.AluOpType.mult

    with tc.tile_pool(name="persist", bufs=1) as PP, \
         tc.tile_pool(name="ft", bufs=2 * NSEG) as FTP, \
         tc.tile_pool(name="acc", bufs=1, space="PSUM") as PA, \
         tc.tile_pool(name="tail", bufs=1, space="PSUM") as TLP, \
         tc.tile_pool(name="dram", bufs=1, space="DRAM") as DP:

        # --- constants (identity over SWDGE: also warms up the Q7 rings
        # long before the collectives need them) ---
        id_f = PP.tile([K, K], F32, name="id_f")
        nc.gpsimd.dma_start(id_f[:], identf_d[:])
        ones19 = PP.tile([K, 1], F32, name="ones19")
        nc.vector.memset(ones19[:], 1.0)
        onesrow = PP.tile([1, K], F32, name="onesrow")
        nc.vector.memset(onesrow[:], 1.0)

        # --- bulk DMA: both streams, interleaved across the two HWDGE
        # queues so stream 0 lands in the first half of the DMA window ---
        qs = [nc.sync, nc.scalar]
        PT = []
        for s in (0, 1):
            pt = PP.tile([128, NCHUNK * K], BF16, name=f"PT{s}")
            qs[s].dma_start(pt[:], pret_d[s][:])
            PT.append(pt)
        fseg = [[], []]
        for s in (0, 1):
            for g in range(NSEG):
                t_ = FTP.tile([128, SEGC * CCW], BF16, name="ftseg")
                qs[g % 2].dma_start(
                    t_[:], ftr_d[s][:, g * SEGC * CCW:(g + 1) * SEGC * CCW])
                fseg[s].append(t_)

        # --- per-stream accumulators ---
        psum_vec = [PA.tile([K, CCW], F32, name=f"pvec{s}") for s in (0, 1)]

        csum = []
        nT = []
        prev_cc = None
        for s in (0, 1):
            # ---- contraction: 128 accumulating matmuls ----
            for g in range(NSEG):
                for t in range(SEGC):
                    i = g * SEGC + t
                    nc.tensor.matmul(
                        psum_vec[s][:],
                        lhsT=PT[s][:, i * K:(i + 1) * K],
                        rhs=fseg[s][g][:, t * CCW:(t + 1) * CCW],
                        start=(i == 0), stop=(i == NCHUNK - 1))

            # ---- stream epilogue (stream 0's overlaps stream 1) ----
            # vec = psum[:, :C] / masksum;  ctx = vec / ||vec||_col
            recip = PP.tile([K, 1], F32, name=f"recip{s}")
            nc.vector.reciprocal(recip[:], psum_vec[s][:, C:C + 1])
            rsq = PP.tile([K, 1], F32, name=f"rsq{s}")
            nc.vector.tensor_mul(rsq[:], recip[:], recip[:])
            sq = PP.tile([K, C], F32, name=f"sq{s}")
            nc.scalar.square(sq[:], psum_vec[s][:, 0:C])
            vec_sb = PP.tile([K, C], F32, name=f"vec_sb{s}")
            nc.vector.tensor_scalar_mul(vec_sb[:], psum_vec[s][:, 0:C],
                                        recip[:])
            # col sums over K of (psum^2 * recip^2) = ||vec||^2
            pn = TLP.tile([1, C], F32, name=f"pn{s}", tag="tlp")
            nc.tensor.matmul(pn[:], lhsT=rsq[:], rhs=sq[:],
                             start=True, stop=True)
            # reference clamps the norm at 1e-12; the norm here is
            # O(1e-2) for non-degenerate input, so the clamp is a no-op.
            nsb = PP.tile([1, C], F32, name=f"nsb{s}")
            nc.scalar.sqrt(nsb[:], pn[:])
            rn = PP.tile([1, C], F32, name=f"rn{s}")
            nc.vector.reciprocal(rn[:], nsb[:])
            # broadcast 1/norm to the K partitions (rank-1 matmul)
            bc = TLP.tile([K, C], F32, name=f"bc{s}", tag="tlp")
            nc.tensor.matmul(bc[:], lhsT=onesrow[:], rhs=rn[:],
                             start=True, stop=True)
            cc_in = PP.tile([K, CCW], F32, name=f"cc_in{s}")
            nc.vector.tensor_mul(cc_in[:, 0:C], vec_sb[:], bc[:])
            # ship the per-core row-mean in the payload (mean over B and
            # mean over C commute)
            xdum = PP.tile([K, C], F32, name=f"xdum{s}")
            nc.scalar.activation(xdum[:], cc_in[:, 0:C],
                                 mybir.ActivationFunctionType.Copy,
                                 scale=1.0 / C,
                                 accum_out=cc_in[:, C:C + 1])

            # ---- AllReduce of the tiny [19,257] payload ----
            b_in = DP.tile([K, CCW], F32, name=f"b_in{s}")
            b_out = DP.tile([K, CCW], F32, name=f"b_out{s}")
            nc.gpsimd.dma_start(b_in[:], cc_in[:])
            cc = nc.gpsimd.collective_compute(
                "AllReduce", add,
                replica_groups=[list(range(n_cores))],
                ins=[b_in.opt()], outs=[b_out.opt()])
            if prev_cc is not None:
                bass._add_dep_helper(
                    cc.ins, prev_cc.ins, sync=False,
                    reason="collectives in stream order")
            prev_cc = cc
            cs = PP.tile([K, CCW], F32, name=f"csum{s}")
            nc.gpsimd.dma_start(cs[:], b_out[:])
            csum.append(cs)

            # ---- side-s Pearson prep (side 0 runs during stream 1;
            # only side 1 trails the last collective) ----
            X = cs[:, 0:C]
            ms = cs[:, C:C + 1]
            xc = PP.tile([K, C], F32, name=f"xc{s}")
            nc.vector.tensor_scalar_sub(xc[:], X, ms)
            xsq = PP.tile([K, C], F32, name=f"xsq{s}")
            ss = PP.tile([K, 1], F32, name=f"ss{s}")
            nc.scalar.activation(xsq[:], xc[:],
                                 mybir.ActivationFunctionType.Square,
                                 accum_out=ss[:])
            sd = PP.tile([K, 1], F32, name=f"sd{s}")
            nc.scalar.sqrt(sd[:], ss[:])
            ri = PP.tile([K, 1], F32, name=f"ri{s}")
            nc.vector.reciprocal(ri[:], sd[:])
            xn = PP.tile([K, C], F32, name=f"xn{s}")
            nc.vector.tensor_scalar(xn[:], X, ms, ri[:],
                                    op0=mybir.AluOpType.subtract,
                                    op1=mult)
            # transpose [K, C] -> [C, K] in two 128-wide blocks
            tps = TLP.tile([128, 2 * K], F32, name=f"tps{s}", tag="tlp")
            for h in (0, 1):
                nc.tensor.matmul(
                    tps[:, h * K:(h + 1) * K],
                    lhsT=xn[:, h * 128:(h + 1) * 128],
                    rhs=id_f[:],
                    is_transpose=True,
                    start=(h == 0), stop=(h == 1))
            nTs = PP.tile([128, 2 * K], F32, name=f"nT{s}")
            nc.vector.tensor_copy(nTs[:], tps[:])
            nT.append(nTs)

        # ---- final correlation ----
        po = TLP.tile([K, K], F32, name="po", tag="tlp")
        for h in (0, 1):
            nc.tensor.matmul(po[:],
                             lhsT=nT[0][:, h * K:(h + 1) * K],
                             rhs=nT[1][:, h * K:(h + 1) * K],
                             start=(h == 0), stop=(h == 1))
        osb = PP.tile([K, K], F32, name="osb")
        nc.vector.tensor_copy(osb[:], po[:])
        nc.sync.dma_start(out_d[:], osb[:])


def build(n_cores=N_CORES):
    nc = bacc.Bacc("TRN2", target_bir_lowering=False, debug=False,
                   enable_asserts=False, num_devices=n_cores)
    pret_d = [nc.dram_tensor(f"pret{s}", [128, NCHUNK * K], BF16,
                             kind="ExternalInput").ap() for s in (1, 2)]
    ftr_d = [nc.dram_tensor(f"ftr{s}", [128, NCHUNK * CCW], BF16,
                            kind="ExternalInput").ap() for s in (1, 2)]
    identf_d = nc.dram_tensor("identf", [K, K], F32, kind="ExternalInput").ap()
    out_d = nc.dram_tensor("out", [K, K], F32, kind="ExternalOutput").ap()
    with tile.TileContext(nc) as tc:
        build_body(nc, tc, pret_d, ftr_d, identf_d, out_d, n_cores)
    nc.compile()
    return nc


_NC_CACHE = {}


def _get_nc():
    if "nc" not in _NC_CACHE:
        _NC_CACHE["nc"] = build(N_CORES)
    return _NC_CACHE["nc"]


class Runner:
    """Executes the compiled Bass program on the first `n_cores` jax
    devices via shard_map, with inputs pre-staged on the devices (the
    analog of the native path's input pre-load in run_neff) so all
    cores start the NEFF near-simultaneously."""

    def __init__(self, nc, n_cores):
        import jax
        from jax.experimental.shard_map import shard_map
        from jax.sharding import Mesh, PartitionSpec, NamedSharding

        bass2jax.install_neuronx_cc_hook()
        self.jax = jax
        self.nc = nc
        self.n_cores = n_cores
        assert nc.dbg_addr is None
        partition_name = (nc.partition_id_tensor.name
                          if nc.partition_id_tensor else None)
        in_names, out_names, out_avals = [], [], []
        for alloc in nc.m.functions[0].allocations:
            if not isinstance(alloc, mybir.MemoryLocationSet):
                continue
            name = alloc.memorylocations[0].name
            if alloc.kind == "ExternalInput":
                if name != partition_name:
                    in_names.append(name)
            elif alloc.kind == "ExternalOutput":
                shape = tuple(alloc.tensor_shape)
                dtype = mybir.dt.np(alloc.dtype)
                out_names.append(name)
                out_avals.append(jax.core.ShapedArray(shape, dtype))
        self.param_names = list(in_names)
        n_params = len(in_names)
        full_in_names = list(in_names) + list(out_names)
        if partition_name is not None:
            full_in_names.append(partition_name)
        full_in_names = tuple(full_in_names)
        donate = tuple(range(n_params, n_params + len(out_names)))
        self.out_names = out_names
        self.out_avals = out_avals

        def _body(*args):
            operands = list(args)
            if partition_name is not None:
                operands.append(bass2jax.partition_id_tensor())
            outs = bass2jax._bass_exec_p.bind(
                *operands,
                out_avals=tuple(out_avals),
                in_names=full_in_names,
                out_names=tuple(out_names),
                lowering_input_output_aliases=(),
                sim_require_finite=True,
                sim_require_nnan=True,
                nc=nc,
            )
            return tuple(outs)

        devices = jax.devices()[:n_cores]
        assert len(devices) == n_cores
        self.mesh = Mesh(np.asarray(devices), ("core",))
        in_specs = (PartitionSpec("core"),) * (n_params + len(out_names))
        out_specs = (PartitionSpec("core"),) * len(out_names)
        self.fn = jax.jit(
            shard_map(_body, mesh=self.mesh, in_specs=in_specs,
                      out_specs=out_specs, check_rep=False),
            donate_argnums=donate, keep_unused=True)
        self.sharding = NamedSharding(self.mesh, PartitionSpec("core"))

    def put(self, in_maps):
        concat = [
            np.concatenate([np.asarray(in_maps[c][n])
                            for c in range(self.n_cores)], axis=0)
            for n in self.param_names
        ]
        arrs = [self.jax.device_put(a, self.sharding) for a in concat]
        self.jax.block_until_ready(arrs)
        return arrs

    def zeros(self):
        zs = [self.jax.device_put(
            np.zeros((self.n_cores * a.shape[0], *a.shape[1:]), a.dtype),
            self.sharding) for a in self.out_avals]
        self.jax.block_until_ready(zs)
        return zs

    def exec(self, dev_in):
        outs = self.fn(*dev_in, *self.zeros())
        self.jax.block_until_ready(outs)
        return {
            name: np.asarray(outs[i]).reshape(
                self.n_cores, *self.out_avals[i].shape)
            for i, name in enumerate(self.out_names)
        }


def _get_runner():
    if "runner" not in _NC_CACHE:
        _NC_CACHE["runner"] = Runner(_get_nc(), N_CORES)
    return _NC_CACHE["runner"]


def make_in_maps(preds1, feats1, preds2, feats2):
    import ml_dtypes
    bf16 = ml_dtypes.bfloat16
    identf = np.eye(K, dtype=np.float32)
    per_stream = {}
    for s, (preds, feats) in enumerate(
            ((preds1, feats1), (preds2, feats2)), start=1):
        # preds [B,K,H,W] -> [B, W(v), H(u), K] -> [B, 128, 128*19]:
        # chunk u's columns are P^T[u*128:(u+1)*128, :19] with the
        # spatial index on partitions
        pr = np.ascontiguousarray(
            preds.astype(bf16).transpose(0, 3, 2, 1)
        ).reshape(B, 128, NCHUNK * K)
        # feats [B,C,H,W] -> [B, W, H, C (+ ones)] -> [B, 128, 128*257]:
        # chunk u is the [w, c] block at h=u, matching pret's chunking;
        # the fused ones column makes psum[:, 256] the mask sums
        ft = np.empty((B, W, H, CCW), dtype=bf16)
        ft[..., :C] = feats.astype(bf16).transpose(0, 3, 2, 1)
        ft[..., C] = 1.0
        per_stream[s] = (pr, ft.reshape(B, 128, NCHUNK * CCW))
    in_maps = []
    for b in range(B):
        in_maps.append({
            "pret1": per_stream[1][0][b],
            "pret2": per_stream[2][0][b],
            "ftr1": per_stream[1][1][b],
            "ftr2": per_stream[2][1][b],
            "identf": identf,
        })
    return in_maps


def kernel(preds1, feats1, preds2, feats2):
    runner = _get_runner()
    in_maps = make_in_maps(preds1, feats1, preds2, feats2)
    dev_in = runner.put(in_maps)
    outs = runner.exec(dev_in)
    return np.asarray(outs["out"][0], dtype=np.float32)
